# revision 20
# baseline (speedup 1.0000x reference)
"""Trainium2 Bass kernel for nn_MicroDVAEModel (vq_codebook).

Sharding: 8 cores = 4 samples x 2 sequence-halves. Per layer the hidden
stream h^T is AllGather'd within each pair (replica groups [[0,1],..]);
K/V are computed from the gathered full-sequence h on every core, Q and
everything else token-local. All activations live transposed [dim, tok]
so no on-device transposes are needed; LayerNorm over the partition dim
uses ones-matmul stats + PE row-broadcasts. Matmuls run in float32r
(tf32-like, full PE rate); elementwise/softmax/LN in fp32. Softmax skips
max-subtraction (scores are bounded) so block accumulation is a plain
concatenation; the key-pad mask is an additive bias inside the exp
activation; 1/denominator is folded in after the V-matmul (extra ones
column on V).
"""

import math
from contextlib import ExitStack

import numpy as np

import concourse.bass as bass
import concourse.tile as tile
from concourse import bacc, mybir
from concourse import bass_utils

B, S, EMBED = 4, 2048, 768
D, H, NL_FULL, DFF = 512, 8, 6, 2048
CODE_DIM, NUM_CODES = 256, 8192
DH = D // H
T = 1024          # local tokens per core
P = 128
NCORES = 8
F32 = mybir.dt.float32
F32R = mybir.dt.float32r
I32 = mybir.dt.int32
SCALE = 1.0 / math.sqrt(DH)
NEG = -1.0e5
GROUPS = [[0, 1], [2, 3], [4, 5], [6, 7]]
AFT = mybir.ActivationFunctionType
ALU = mybir.AluOpType
ts = bass.ts


def build(nl=NL_FULL):
    nc = bacc.Bacc("TRN2", target_bir_lowering=False, debug=False,
                   num_devices=NCORES)
    NLN = 2 * nl + 2

    def din(name, shape, dt=F32R):
        return nc.dram_tensor(name, shape, dt, kind="ExternalInput")

    xT = din("xT", (EMBED, T))
    peT = din("peT", (D, T), F32)
    maskb = din("maskb", (P, S // P), F32)
    mtok = din("mtok", (P, T // P), F32)
    winT = din("winT", (EMBED, D))
    wqkT = din("wqkT", (nl, D, 2 * D))
    bqk = din("bqk", (nl, P, 2 * D // P), F32)
    wvT = din("wvT", (nl, D, D))
    bv = din("bv", (nl, 1, D))
    onesr = din("onesr", (1, P))
    woT = din("woT", (nl, D, D))
    w1T = din("w1T", (nl, D, DFF))
    b1c = din("b1c", (nl, P, DFF // P), F32)
    w2T = din("w2T", (nl, DFF, D))
    lnS = din("lnS", (NLN, D, 2), F32)    # cols [ones, 2*bo]
    lnGB = din("lnGB", (NLN, 4, D), F32)  # rows [g, b, g*bo, -g]
    lnC = din("lnC", (NLN, 1, 2), F32)    # [sum bo, sum bo^2]
    wpvT = din("wpvT", (D, CODE_DIM))
    cnT = din("cnT", (CODE_DIM, NUM_CODES))
    iotad = din("iotad", (P, NUM_CODES), F32)

    zT_out = nc.dram_tensor("zT", (CODE_DIM, T), F32R, kind="ExternalOutput")
    idx_out = nc.dram_tensor("idx", (P, T // P), I32, kind="ExternalOutput")

    with ExitStack() as top:
        tc = top.enter_context(tile.TileContext(nc))
        glob = top.enter_context(tc.tile_pool(name="glob", bufs=1))
        psA = top.enter_context(tc.tile_pool(name="psA", bufs=3, space="PSUM"))
        dram = top.enter_context(tc.tile_pool(name="dram", bufs=2,
                                              space="DRAM"))

        maskb_sb = glob.tile([P, S // P], F32, tag="maskb")
        nc.sync.dma_start(maskb_sb[:], maskb.ap())
        mtok_sb = glob.tile([P, T // P], F32, tag="mtok")
        nc.sync.dma_start(mtok_sb[:], mtok.ap())
        onesr_sb = glob.tile([1, P], F32R, tag="onesr")
        nc.sync.dma_start(onesr_sb[:], onesr.ap())
        ones32 = glob.tile([1, P], F32, tag="ones32")
        nc.vector.memset(ones32[:], 1.0)
        eps_sb = glob.tile([1, 1], F32, tag="eps")
        nc.vector.memset(eps_sb[:], 1e-5)

        def layernorm(out_pool, x_chunks, ln_i, dval, out_tag, out_bufs=1):
            """x_chunks: list of fp32 APs [P, T] (partition = feature dims).
            Returns normalized F32R chunk tiles [P, T] from out_pool."""
            nch = len(x_chunks)
            out = []
            with tc.tile_pool(name=f"ln{ln_i}", bufs=1) as pool, \
                    tc.tile_pool(name=f"psS{ln_i}", bufs=1,
                                 space="PSUM") as psS:
                lnS_sb = pool.tile([P, nch, 64], F32, tag="lnS_sb")
                nc.vector.memset(lnS_sb[:], 0.0)
                nc.sync.dma_start(
                    lnS_sb[:, :, 0:1],
                    lnS.ap()[ln_i, : nch * P, 0:1].rearrange(
                        "(c p) k -> p c k", p=P))
                nc.sync.dma_start(
                    lnS_sb[:, :, 32:33],
                    lnS.ap()[ln_i, : nch * P, 1:2].rearrange(
                        "(c p) k -> p c k", p=P))
                lnG_sb = pool.tile([1, D], F32, tag="lnG_sb")
                nc.sync.dma_start(lnG_sb[:, : nch * P],
                                  lnGB.ap()[ln_i, 0:1, : nch * P])
                lnB_sb = pool.tile([3, D], F32, tag="lnB_sb")
                nc.sync.dma_start(lnB_sb[:, : nch * P],
                                  lnGB.ap()[ln_i, 1:4, : nch * P])
                lnC_sb = pool.tile([1, 2], F32, tag="lnC_sb")
                nc.sync.dma_start(lnC_sb[:], lnC.ap()[ln_i])

                sq = []
                for ic in range(nch):
                    s = pool.tile([P, T], F32, tag=f"sq{ic}", name=f"sq{ic}")
                    nc.vector.tensor_tensor(s[:], x_chunks[ic], x_chunks[ic],
                                            op=ALU.mult)
                    sq.append(s)
                cpS1 = pool.tile([1, T], F32, tag="cpS1")
                cpS2 = pool.tile([1, T], F32, tag="cpS2")
                cpQ = pool.tile([1, T], F32, tag="cpQ")
                for th in range(2):
                    cs = ts(th, 512)
                    sA = psS.tile([32, 512], F32, tag="sA", name="sA")
                    sB = psS.tile([32, 512], F32, tag="sB", name="sB")
                    sC = psS.tile([32, 512], F32, tag="sC", name="sC")
                    for ic in range(nch):
                        nc.tensor.matmul(sA[:], lnS_sb[:, ic, 0:32],
                                         x_chunks[ic][:, cs],
                                         start=(ic == 0),
                                         stop=(ic == nch - 1))
                    for ic in range(nch):
                        nc.tensor.matmul(sB[:], lnS_sb[:, ic, 32:64],
                                         x_chunks[ic][:, cs],
                                         start=(ic == 0),
                                         stop=(ic == nch - 1))
                    for ic in range(nch):
                        nc.tensor.matmul(sC[:], lnS_sb[:, ic, 0:32],
                                         sq[ic][:, cs],
                                         start=(ic == 0),
                                         stop=(ic == nch - 1))
                    nc.scalar.activation(cpS1[:, cs], sA[0:1, :], AFT.Copy)
                    nc.scalar.activation(cpS2[:, cs], sB[0:1, :], AFT.Copy)
                    nc.scalar.activation(cpQ[:, cs], sC[0:1, :], AFT.Copy)
                rows = pool.tile([3, T], F32, tag="lnrows")
                nc.vector.memset(rows[0:1, :], 1.0)
                mu = pool.tile([1, T], F32, tag="lnmu")
                nc.vector.tensor_scalar(mu[:], cpS1[:], lnC_sb[:, 0:1],
                                        1.0 / dval, op0=ALU.add, op1=ALU.mult)
                nc.vector.tensor_tensor(cpQ[:], cpQ[:], cpS2[:], op=ALU.add)
                nc.vector.tensor_scalar(cpQ[:], cpQ[:], lnC_sb[:, 1:2],
                                        1.0 / dval, op0=ALU.add, op1=ALU.mult)
                nc.vector.tensor_tensor(cpS2[:], mu[:], mu[:], op=ALU.mult)
                nc.vector.tensor_tensor(cpQ[:], cpQ[:], cpS2[:],
                                        op=ALU.subtract)
                nc.scalar.activation(cpS1[:], cpQ[:], AFT.Sqrt,
                                     bias=eps_sb[:])
                rrow = pool.tile([1, T], F32, tag="lnrrow")
                nc.vector.reciprocal(rrow[:], cpS1[:])
                nc.vector.tensor_tensor(mu[:], mu[:], rrow[:], op=ALU.mult)
                nc.sync.dma_start(rows[1:2, :], rrow[:])
                nc.sync.dma_start(rows[2:3, :], mu[:])
                for ic in range(nch):
                    o = out_pool.tile([P, T], F32R, tag=f"{out_tag}{ic}",
                                      name=f"{out_tag}{ic}", bufs=out_bufs)
                    for th in range(2):
                        cs = ts(th, 512)
                        a_ps = psA.tile([P, 512], F32, tag="psA")
                        nc.tensor.matmul(a_ps[:], lnG_sb[0:1, ts(ic, P)],
                                         rrow[:, cs], start=True, stop=True)
                        b_ps = psA.tile([P, 512], F32, tag="psA")
                        nc.tensor.matmul(b_ps[:], lnB_sb[:, ts(ic, P)],
                                         rows[:, cs], start=True, stop=True)
                        tmp = pool.tile([P, 512], F32, tag="lntmp", bufs=2)
                        nc.vector.tensor_tensor(tmp[:], x_chunks[ic][:, cs],
                                                a_ps[:], op=ALU.mult)
                        nc.vector.tensor_tensor(o[:, cs], tmp[:], b_ps[:],
                                                op=ALU.add)
                    out.append(o)
            return out

        # ---------------- input projection ----------------
        h = []
        with tc.tile_pool(name="inp", bufs=1) as inp:
            xin = []
            for ic in range(EMBED // P):
                t = inp.tile([P, T], F32R, tag=f"xin{ic}")
                nc.sync.dma_start(t[:], xT.ap()[ts(ic, P), :])
                xin.append(t)
            win_sb = []
            for ic in range(EMBED // P):
                t = inp.tile([P, D], F32R, tag=f"win{ic}")
                nc.sync.dma_start(t[:], winT.ap()[ts(ic, P), :])
                win_sb.append(t)
            for oc in range(D // P):
                pe_t = inp.tile([P, T], F32, tag=f"pe{oc}")
                nc.sync.dma_start(pe_t[:], peT.ap()[ts(oc, P), :])
                ht = glob.tile([P, T], F32R, tag=f"h{oc}", bufs=2,
                               name=f"h{oc}")
                for th in range(2):
                    cs = ts(th, 512)
                    ps = psA.tile([P, 512], F32, tag="psA")
                    for ic in range(EMBED // P):
                        nc.tensor.matmul(ps[:], win_sb[ic][:, ts(oc, P)],
                                         xin[ic][:, cs], start=(ic == 0),
                                         stop=(ic == EMBED // P - 1))
                    nc.vector.tensor_tensor(ht[:, cs], ps[:], pe_t[:, cs],
                                            op=ALU.add)
                h.append(ht)

        # ---------------- encoder layers ----------------
        for l in range(nl):
            # AllGather h within the pair
            hb_in = dram.tile([D, T], F32R, tag="hb_in")
            hb_out = dram.tile([2 * D, T], F32R, tag="hb_out")
            for ic in range(4):
                nc.sync.dma_start(hb_in[ts(ic, P), :], h[ic][:])
            nc.gpsimd.collective_compute(
                "AllGather", ALU.bypass, replica_groups=GROUPS,
                ins=[hb_in.opt()], outs=[hb_out.opt()])

            with tc.tile_pool(name=f"lp{l}", bufs=1) as lp:
                b1_sb = lp.tile([P, DFF // P], F32, tag="b1_sb")
                nc.sync.dma_start(b1_sb[:], b1c.ap()[l])

                with tc.tile_pool(name=f"atp{l}", bufs=1) as atp:
                    qT = [atp.tile([P, T], F32R, tag=f"qT{i}", name=f"qT{i}")
                          for i in range(4)]
                    kT = [atp.tile([P, S], F32R, tag=f"kT{i}", name=f"kT{i}")
                          for i in range(4)]
                    vsb = [atp.tile([P, 8, 65], F32R, tag=f"vsb{i}",
                                    name=f"vsb{i}") for i in range(16)]

                    with tc.tile_pool(name=f"kvp{l}", bufs=1) as kvp:
                        bqk_sb = kvp.tile([P, 8], F32, tag="bqk_sb")
                        nc.sync.dma_start(bqk_sb[:], bqk.ap()[l])
                        bv_sb = kvp.tile([1, D], F32R, tag="bv_sb")
                        nc.sync.dma_start(bv_sb[:], bv.ap()[l])
                        wqk_sb = []
                        for ic in range(4):
                            t = kvp.tile([P, 2 * D], F32R, tag=f"wqk{ic}")
                            nc.sync.dma_start(t[:],
                                              wqkT.ap()[l, ts(ic, P), :])
                            wqk_sb.append(t)
                        wv_sb = []
                        for ic in range(4):
                            t = kvp.tile([P, D], F32R, tag=f"wv{ic}")
                            nc.sync.dma_start(t[:], wvT.ap()[l, ts(ic, P), :])
                            wv_sb.append(t)
                        # Q from local h (overlaps with the AllGather)
                        for oc in range(4):
                            for th in range(2):
                                cs = ts(th, 512)
                                ps = psA.tile([P, 512], F32, tag="psA")
                                for ic in range(4):
                                    nc.tensor.matmul(
                                        ps[:], wqk_sb[ic][:, ts(oc, P)],
                                        h[ic][:, cs],
                                        start=(ic == 0), stop=(ic == 3))
                                nc.vector.tensor_scalar(
                                    qT[oc][:, cs], ps[:],
                                    bqk_sb[:, oc:oc + 1], None, op0=ALU.add)
                        # K and V for all 2048 tokens, streamed per tq
                        for tq in range(4):
                            hf = tq // 2
                            cs_g = ts(tq % 2, 512)
                            hfc = []
                            for ic in range(4):
                                t = kvp.tile([P, 512], F32R,
                                             tag=f"hfc{ic}", bufs=2,
                                             name=f"hfc{ic}")
                                nc.sync.dma_start(
                                    t[:],
                                    hb_out[hf * D + ic * P:
                                           hf * D + (ic + 1) * P, cs_g])
                                hfc.append(t)
                            for oc in range(4):
                                ps = psA.tile([P, 512], F32, tag="psA")
                                for ic in range(4):
                                    nc.tensor.matmul(
                                        ps[:], wqk_sb[ic][:, ts(4 + oc, P)],
                                        hfc[ic][:], start=(ic == 0),
                                        stop=(ic == 3))
                                nc.vector.tensor_scalar(
                                    kT[oc][:, ts(tq, 512)], ps[:],
                                    bqk_sb[:, 4 + oc:5 + oc], None,
                                    op0=ALU.add)
                            for kk in range(4):
                                kc = 4 * tq + kk
                                ps = psA.tile([P, 512], F32, tag="psA")
                                for ic in range(4):
                                    nc.tensor.matmul(
                                        ps[:], hfc[ic][:, ts(kk, P)],
                                        wv_sb[ic][:], start=(ic == 0),
                                        stop=False)
                                nc.tensor.matmul(ps[:], onesr_sb[:],
                                                 bv_sb[:], start=False,
                                                 stop=True)
                                psv = ps[:].rearrange("p (a b) -> p a b",
                                                      b=64)
                                nc.vector.tensor_copy(vsb[kc][:, :, 0:64],
                                                      psv)
                                nc.vector.memset(vsb[kc][:, :, 64:65].bitcast(F32), 1.0)

                    # attention (+ fused Wo/residual per th-half)
                    x1 = [lp.tile([P, T], F32, tag=f"x1{i}", name=f"x1{i}")
                          for i in range(4)]
                    with tc.tile_pool(name=f"att{l}", bufs=1) as att, \
                            tc.tile_pool(name=f"psC{l}", bufs=2,
                                         space="PSUM") as psC:
                        wo_sb = []
                        for ic in range(4):
                            t = att.tile([P, D], F32R, tag=f"wo{ic}")
                            nc.sync.dma_start(t[:], woT.ap()[l, ts(ic, P), :])
                            wo_sb.append(t)
                        for th in range(2):
                            cs = ts(th, 512)
                            ctxn = [att.tile([P, 512], F32R,
                                             tag=f"ctxn{i}", bufs=2,
                                             name=f"ctxn{i}")
                                    for i in range(4)]
                            for hp in range(4):
                                cps = [psC.tile([65, 512], F32, tag="ctxps",
                                                name="ctxps")
                                       for _ in range(2)]
                                for kc in range(16):
                                    for sub in range(2):
                                        hh = 2 * hp + sub
                                        rs = slice(64 * sub, 64 * (sub + 1))
                                        sps = psA.tile([P, 512], F32,
                                                       tag="psA")
                                        nc.tensor.matmul(
                                            sps[:], kT[hp][rs, ts(kc, P)],
                                            qT[hp][rs, cs],
                                            start=True, stop=True)
                                        E = att.tile([P, 512], F32R,
                                                     tag=f"E{sub}", bufs=2)
                                        nc.scalar.activation(
                                            E[:], sps[:], AFT.Exp,
                                            bias=maskb_sb[:, kc:kc + 1],
                                            scale=SCALE)
                                        nc.tensor.matmul(
                                            cps[sub][:], vsb[kc][:, hh, :],
                                            E[:], start=(kc == 0),
                                            stop=(kc == 15))
                                for sub in range(2):
                                    dsb = att.tile([1, 512], F32,
                                                   tag=f"dsb{sub}", bufs=1)
                                    nc.vector.reciprocal(
                                        dsb[:], cps[sub][64:65, :])
                                    bc = psA.tile([P, 512], F32, tag="psA")
                                    nc.tensor.matmul(bc[0:64, :],
                                                     ones32[:, 0:64],
                                                     dsb[:], start=True,
                                                     stop=True)
                                    bcs = att.tile([64, 512], F32,
                                                   tag=f"bcs{sub}", bufs=1)
                                    nc.scalar.activation(bcs[:],
                                                         bc[0:64, :],
                                                         AFT.Copy)
                                    nc.vector.tensor_tensor(
                                        ctxn[hp][64 * sub:64 * (sub + 1), :],
                                        cps[sub][0:64, :], bcs[:],
                                        op=ALU.mult)
                            # Wo + residual for this th-half
                            for oc in range(4):
                                ps = psA.tile([P, 512], F32, tag="psA")
                                for dc in range(4):
                                    nc.tensor.matmul(ps[:],
                                                     wo_sb[dc][:, ts(oc, P)],
                                                     ctxn[dc][:],
                                                     start=(dc == 0),
                                                     stop=(dc == 3))
                                nc.vector.tensor_tensor(
                                    x1[oc][:, cs], ps[:],
                                    h[oc][:, cs].bitcast(F32), op=ALU.add)
                hm = layernorm(lp, [x[:] for x in x1], 2 * l, float(D), "hm")

                # FFN
                x2 = [lp.tile([P, T], F32, tag=f"x2{i}", name=f"x2{i}")
                      for i in range(4)]
                with tc.tile_pool(name=f"ffn{l}", bufs=1) as ffn, \
                        tc.tile_pool(name=f"psF{l}", bufs=4,
                                     space="PSUM") as psF:
                    for th in range(2):
                        cs = ts(th, 512)
                        gsb = []
                        for gc in range(16):
                            g = ffn.tile([P, 512], F32R, tag=f"g{gc}",
                                         name=f"g{gc}")
                            ps = psA.tile([P, 512], F32, tag="psA")
                            for ic in range(4):
                                w1t = ffn.tile([P, P], F32R,
                                               tag=f"w1t{(gc * 4 + ic) % 8}",
                                               name="w1t")
                                nc.sync.dma_start(
                                    w1t[:],
                                    w1T.ap()[l, ts(ic, P), ts(gc, P)])
                                nc.tensor.matmul(ps[:], w1t[:],
                                                 hm[ic][:, cs],
                                                 start=(ic == 0),
                                                 stop=(ic == 3))
                            nc.scalar.activation(g[:], ps[:], AFT.Gelu,
                                                 bias=b1_sb[:, gc:gc + 1])
                            gsb.append(g)
                        fps = [psF.tile([P, 512], F32, tag="psF",
                                        name="psF") for _ in range(4)]
                        for gc in range(16):
                            w2t = ffn.tile([P, D], F32R, tag=f"w2t{gc % 3}",
                                           name="w2t")
                            nc.sync.dma_start(w2t[:],
                                              w2T.ap()[l, ts(gc, P), :])
                            for oc in range(4):
                                nc.tensor.matmul(fps[oc][:],
                                                 w2t[:, ts(oc, P)],
                                                 gsb[gc][:],
                                                 start=(gc == 0),
                                                 stop=(gc == 15))
                        for oc in range(4):
                            nc.vector.tensor_tensor(
                                x2[oc][:, cs], fps[oc][:],
                                hm[oc][:, cs].bitcast(F32), op=ALU.add)
                h = layernorm(glob, [x[:] for x in x2], 2 * l + 1, float(D),
                              "h", out_bufs=2)

        # ---------------- final LNs + z projection + VQ ----------------
        zt = None
        with tc.tile_pool(name="fin2", bufs=1) as fin2:
            with tc.tile_pool(name="fin1", bufs=1) as fin1:
                henc = layernorm(fin1, [x[:].bitcast(F32) for x in h],
                                 2 * nl, float(D), "henc")
                wpv_sb = []
                for ic in range(4):
                    t = fin1.tile([P, CODE_DIM], F32R, tag=f"wpv{ic}")
                    nc.sync.dma_start(t[:], wpvT.ap()[ts(ic, P), :])
                    wpv_sb.append(t)
                zraw = [fin1.tile([P, T], F32, tag=f"zraw{i}",
                                  name=f"zraw{i}") for i in range(2)]
                for oc in range(2):
                    for th in range(2):
                        cs = ts(th, 512)
                        ps = psA.tile([P, 512], F32, tag="psA")
                        for ic in range(4):
                            nc.tensor.matmul(ps[:], wpv_sb[ic][:, ts(oc, P)],
                                             henc[ic][:, cs],
                                             start=(ic == 0), stop=(ic == 3))
                        nc.scalar.activation(zraw[oc][:, cs], ps[:],
                                             AFT.Copy)
                zt = layernorm(fin2, [z[:] for z in zraw], 2 * nl + 1,
                               float(CODE_DIM), "zt")
            for oc in range(2):
                nc.sync.dma_start(zT_out.ap()[ts(oc, P), :], zt[oc][:])

            iota_sb = fin2.tile([P, NUM_CODES], F32, tag="iota")
            nc.sync.dma_start(iota_sb[:], iotad.ap())
            idxf = fin2.tile([P, T // P], F32, tag="idxf")
            for tcn in range(T // P):
                lg = fin2.tile([P, NUM_CODES], F32, tag="lg", name="lg")
                for cc in range(16):
                    ps = psA.tile([P, 512], F32, tag="psA")
                    for jc in range(2):
                        cnt = fin2.tile([P, 512], F32R,
                                        tag=f"cnt{(2 * cc + jc) % 4}",
                                        name="cnt")
                        nc.sync.dma_start(cnt[:],
                                          cnT.ap()[ts(jc, P), ts(cc, 512)])
                        nc.tensor.matmul(ps[:], zt[jc][:, ts(tcn, P)],
                                         cnt[:],
                                         start=(jc == 0), stop=(jc == 1))
                    nc.scalar.activation(lg[:, ts(cc, 512)], ps[:],
                                         AFT.Copy)
                mx8 = fin2.tile([P, 8], F32, tag=f"mx8{tcn % 2}")
                nc.vector.max(out=mx8[:], in_=lg[:])
                nc.gpsimd.tensor_scalar(lg[:], lg[:], mx8[:, 0:1], None,
                                        op0=ALU.is_ge)
                nc.vector.tensor_tensor(lg[:], lg[:], iota_sb[:],
                                        op=ALU.mult)
                mr = fin2.tile([P, 1], F32, tag=f"mr{tcn % 2}")
                nc.vector.tensor_reduce(out=mr[:], in_=lg[:], op=ALU.max,
                                        axis=mybir.AxisListType.X)
                nc.vector.tensor_scalar(idxf[:, tcn:tcn + 1], mr[:], -1.0,
                                        float(NUM_CODES), op0=ALU.mult,
                                        op1=ALU.add)
            nc.vector.tensor_tensor(idxf[:], idxf[:], mtok_sb[:],
                                    op=ALU.mult)
            idxi = fin2.tile([P, T // P], I32, tag="idxi")
            nc.vector.tensor_copy(idxi[:], idxf[:])
            nc.sync.dma_start(idx_out.ap(), idxi[:])

    nc.compile()
    return nc


# ---------------------------------------------------------------------------
# host side
# ---------------------------------------------------------------------------
_CACHE = {}


def _get_nc(nl):
    if nl not in _CACHE:
        _CACHE[nl] = build(nl)
    return _CACHE[nl]


def _sinusoid_pe(seq_len, d_model):
    pos = np.arange(seq_len, dtype=np.float32)[:, None]
    div = np.exp(np.arange(0, d_model, 2, dtype=np.float32)
                 * (-math.log(10000.0) / d_model)).astype(np.float32)
    pe = np.zeros((seq_len, d_model), dtype=np.float32)
    pe[:, 0::2] = np.sin(pos * div)
    pe[:, 1::2] = np.cos(pos * div)
    return pe


def _c(a):
    return np.ascontiguousarray(a, dtype=np.float32)


def prep_inputs(x, mask, params, nl=NL_FULL):
    x = np.asarray(x, np.float32)
    mask = np.asarray(mask, np.int32)
    p = {k: np.asarray(v, np.float32) for k, v in params.items()}
    NLN = 2 * nl + 2

    common = {}
    common["winT"] = _c(p["W_in"].T)
    common["wqkT"] = _c(np.stack([p["Wqkv"][l][:2 * D].T for l in range(nl)]))
    common["bqk"] = _c(np.stack(
        [p["bqkv"][l][:2 * D].reshape(8, P).T for l in range(nl)]))
    common["wvT"] = _c(np.stack([p["Wqkv"][l][2 * D:].T for l in range(nl)]))
    common["bv"] = _c(np.stack(
        [p["bqkv"][l][2 * D:][None, :] for l in range(nl)]))
    common["onesr"] = np.ones((1, P), np.float32)
    common["woT"] = _c(np.stack([p["Wo"][l].T for l in range(nl)]))
    common["w1T"] = _c(np.stack([p["W1"][l].T for l in range(nl)]))
    common["b1c"] = _c(np.stack(
        [p["b1"][l].reshape(DFF // P, P).T for l in range(nl)]))
    common["w2T"] = _c(np.stack([p["W2"][l].T for l in range(nl)]))

    lnS = np.zeros((NLN, D, 2), np.float32)
    lnGB = np.zeros((NLN, 4, D), np.float32)
    lnC = np.zeros((NLN, 1, 2), np.float32)

    def set_ln(i, g, b, bo, dim):
        lnS[i, :dim, 0] = 1.0
        lnS[i, :dim, 1] = 2.0 * bo
        lnGB[i, 0, :dim] = g
        lnGB[i, 1, :dim] = b
        lnGB[i, 2, :dim] = g * bo
        lnGB[i, 3, :dim] = -g
        lnC[i, 0, 0] = bo.sum()
        lnC[i, 0, 1] = (bo ** 2).sum()

    for l in range(nl):
        set_ln(2 * l, p["g1"][l], p["bln1"][l], p["bo"][l], D)
        set_ln(2 * l + 1, p["g2"][l], p["bln2"][l], p["b2"][l], D)
    set_ln(2 * nl, p["g_enc"], p["b_enc"], np.zeros(D, np.float32), D)
    set_ln(2 * nl + 1, p["g_pv"], p["bln_pv"], p["b_pv"], CODE_DIM)
    common["lnS"], common["lnGB"], common["lnC"] = lnS, lnGB, lnC

    common["wpvT"] = _c(p["W_pv"].T)
    cb = p["codebook"]
    cn = cb / np.maximum(np.linalg.norm(cb, axis=1, keepdims=True), 1e-8)
    common["cnT"] = _c(cn.T)
    common["iotad"] = _c(np.broadcast_to(
        (NUM_CODES - np.arange(NUM_CODES, dtype=np.float32))[None, :],
        (P, NUM_CODES)))

    pe = _sinusoid_pe(S, D)
    peT_full = pe.T + p["b_in"][:, None]

    in_maps = []
    for c in range(NCORES):
        b, hf = c // 2, c % 2
        m = dict(common)
        m["xT"] = _c(x[b, hf * T:(hf + 1) * T].T)
        m["peT"] = _c(peT_full[:, hf * T:(hf + 1) * T])
        mb = np.where(mask[b] == 0, NEG, 0.0).astype(np.float32)
        m["maskb"] = _c(mb.reshape(S // P, P).T)
        m["mtok"] = _c(mask[b, hf * T:(hf + 1) * T].astype(np.float32)
                       .reshape(T // P, P).T)
        in_maps.append(m)
    return in_maps


def assemble(results):
    indices = np.zeros((B, S), np.int32)
    z = np.zeros((B, S, CODE_DIM), np.float32)
    for c in range(NCORES):
        b, hf = c // 2, c % 2
        r = results[c]
        z[b, hf * T:(hf + 1) * T] = r["zT"].T
        indices[b, hf * T:(hf + 1) * T] = r["idx"].T.reshape(T)
    return indices, z


def run(x, mask, params, nl=NL_FULL, trace=False, tmpdir=None):
    nc = _get_nc(nl)
    in_maps = prep_inputs(x, mask, params, nl)
    res = bass_utils.run_bass_kernel_spmd(
        nc, in_maps, core_ids=list(range(NCORES)), trace=trace, tmpdir=tmpdir)
    return assemble(res.results), res


def kernel(x, mask, params):
    (indices, z), _ = run(x, mask, params)
    return indices, z
